# revision 28
# baseline (speedup 1.0000x reference)
"""Single-head attention (B=4, S=2048, D=1024) on 8 trn2 NeuronCores.

Sharding: core = batch*2 + kv_half.  Each core receives ONLY its kv-half
of x^T (xh = x[b].T[:, half]) and computes
  Qown = xh @ Wq^T + bq        (its 1024 own-half queries)
  K    = xh @ Wk^T + bk        (its 1024 keys)
  V    = xh @ Wv^T             (bv folded in on host)
The pair cores exchange Q^T halves with a pairwise HBM AllGather (2MB)
that overlaps the K/V projections; each core then attends all 2048
queries against its kv-half:
  ST = K @ Q^T, PT = exp(ST/32), l = ones^T @ sum_j PT_j, accT = V' @ PT
Queries are processed in LOCAL order ([own half | peer half]) so the ST
pass starts without waiting on the collective; the host swaps the odd
cores' output halves back and merges:
  out[b] = (acc0 + acc1) / (l0 + l1) + bv.

All tensors are bf16 on device (fp32 PSUM); bf16 matmuls run at full PE
rate and everything stays SBUF-resident.  DMAs are issued on the sync
ring only (ACT-ring DMAs wedge the device) and are emitted in deadline
order — the SP engine processes them serially at ~185GB/s, so program
order is the schedule.  The peer-half gather-in DMAs read
qall_d[1 - parity] through a runtime register index (bass.ts) so one
NEFF serves both pair members.

Perf notes vs the first working version (224.2us):
 - weight DMAs are one [128,1024] burst per row-block (2KB/partition);
 - bq/bk ride in a single packed [128,16] f32 DMA placed late enough
   not to delay phase-A x/w tiles but before the first PSUM drain;
 - only 5 PE warmup matmuls: phase A itself ramps the clock inside the
   DMA-starved window instead of junk work delaying it;
 - l(blk) = ones^T @ (tree-sum of the 8 PT tiles on DVE): 1 PE matmul
   per block instead of 8 (-14K PE cycles);
 - one PSUM pool for all phases (no pool-transition barrier);
 - the last accT drain is split in half so the tail cast+DMA pipeline
   drains in smaller quanta.
"""

import sys
import numpy as np

for _p in ("/root/.axon_site/_ro/trn_rl_repo", "/opt/trn_rl_repo"):
    if _p not in sys.path:
        sys.path.append(_p)

import ml_dtypes
import concourse.bass as bass
import concourse.tile as tile
from concourse import bacc, mybir
from concourse.bass_utils import run_bass_kernel_spmd

F32 = mybir.dt.float32
BF16 = mybir.dt.bfloat16
BF = ml_dtypes.bfloat16

B, S, D = 4, 2048, 1024
H = S // 2          # kv-half size (1024)
DT = D // 128       # 8 contraction tiles
ET = D // 128       # 8 output-dim tiles
XCH = H // 512      # 2 column chunks of xh
NCH = S // 512      # 4 query chunks
N_CORES = 8

_compiled = None


def _build():
    nc = bacc.Bacc("TRN2", target_bir_lowering=False, debug=False,
                   num_devices=N_CORES)

    xh = nc.dram_tensor("xh", [D, H], BF16, kind="ExternalInput").ap()
    wqt = nc.dram_tensor("wqt", [D, D], BF16, kind="ExternalInput").ap()
    wkt = nc.dram_tensor("wkt", [D, D], BF16, kind="ExternalInput").ap()
    wvt = nc.dram_tensor("wvt", [D, D], BF16, kind="ExternalInput").ap()
    bqk = nc.dram_tensor("bqk", [128, 2 * ET], F32, kind="ExternalInput").ap()
    ones = nc.dram_tensor("ones", [128, 1], BF16, kind="ExternalInput").ap()

    accT_d = nc.dram_tensor("accT_d", [D, S], BF16, kind="ExternalOutput").ap()
    l_d = nc.dram_tensor("l_d", [S], F32, kind="ExternalOutput").ap()

    Ident = mybir.ActivationFunctionType.Identity
    Exp = mybir.ActivationFunctionType.Exp

    with tile.TileContext(nc) as tc:
        with (
            tc.tile_pool(name="const", bufs=1) as const,
            tc.tile_pool(name="wpool", bufs=1) as wpool,
            tc.tile_pool(name="xpool", bufs=1) as xpool,
            tc.tile_pool(name="qtp", bufs=1) as qtp,
            tc.tile_pool(name="ktp", bufs=1) as ktp,
            tc.tile_pool(name="vvp", bufs=1) as vvp,
            tc.tile_pool(name="ptp", bufs=1) as ptp,
            tc.tile_pool(name="ltp", bufs=2) as ltp,
            tc.tile_pool(name="stg", bufs=4) as stg,
            tc.tile_pool(name="lst", bufs=2) as lst,
        ):
            ones_sb = const.tile([128, 1], BF16, name="ones_sb")
            bqk_sb = const.tile([128, 2 * ET], F32, name="bqk_sb")

            wqa = wpool.tile([128, DT, D], BF16, name="wqa")
            wka = wpool.tile([128, DT, D], BF16, name="wka")
            wva = wpool.tile([128, DT, D], BF16, name="wva")
            xa = [xpool.tile([128, DT, 512], BF16, name=f"xa{c}")
                  for c in range(XCH)]
            qt_c = [qtp.tile([128, ET, 512], BF16, name=f"qt{c}")
                    for c in range(NCH)]
            kt_c = [ktp.tile([128, ET, 512], BF16, name=f"kt{cc}")
                    for cc in range(XCH)]
            v_c = [vvp.tile([128, 4, D], BF16, name=f"v{cc}")
                   for cc in range(XCH)]
            pt_b = [ptp.tile([128, 8, 512], BF16, name=f"pt{blk}")
                    for blk in range(NCH)]

            # DRAM bounce buffers for the pairwise Q AllGather, split per
            # 512-query chunk so the first exchange starts ~15us earlier
            with tc.tile_pool(name="dram", bufs=1, space="DRAM") as dram:
                qown_c = [dram.tile([D, 512], BF16, name=f"qown{c}")
                          for c in range(XCH)]
                qall_c = [dram.tile([2, D, 512], BF16, name=f"qall{c}")
                          for c in range(XCH)]

            # core parity (runtime register) for the predicated gather-in
            parity = nc.sync.partition_id() & 1
            not_parity = 1 - parity

            # ---- input DMAs in deadline order (transfers run at
            # ~180GB/s effective aggregate regardless of burst size, so
            # per-row-block granularity minimizes first-use latency;
            # program order IS the schedule) ----
            def dma_w(dst, src, dt):
                nc.sync.dma_start(
                    out=dst[:, dt, :],
                    in_=src[dt * 128:(dt + 1) * 128, :])

            def dma_x(c, dt):
                nc.sync.dma_start(
                    out=xa[c][:, dt, :],
                    in_=xh[dt * 128:(dt + 1) * 128, c * 512:(c + 1) * 512])

            # first-needed pieces lead the stream in consumption order;
            # the packed const DMA slots in after dt5 (needed at the first
            # PSUM drain, which happens at dt7 of chunk 0).  wq's dt0 rows
            # go in two column-halves so the very first matmul chains
            # (e0-3 of dt0) only wait on a 128KB piece, not 256KB.
            nc.sync.dma_start(out=wqa[:, 0, 0:512],
                              in_=wqt[0:128, 0:512])
            dma_x(0, 0)
            nc.sync.dma_start(out=wqa[:, 0, 512:1024],
                              in_=wqt[0:128, 512:1024])
            for dt in range(1, 6):
                dma_w(wqa, wqt, dt)
                dma_x(0, dt)
            nc.sync.dma_start(out=ones_sb, in_=ones)
            nc.sync.dma_start(out=bqk_sb, in_=bqk)
            for dt in range(6, DT):
                dma_w(wqa, wqt, dt)
                dma_x(0, dt)
            for dt in range(DT):
                dma_x(1, dt)
            for dt in range(DT):
                dma_w(wka, wkt, dt)

            with tc.tile_pool(name="ps", bufs=8, space="PSUM") as psp:
                # ---- PE warmup: junk matmuls kick the PE clock ramp
                # while the first input DMAs land; phase A then continues
                # the ramp inside its DMA-paced window.  The memset runs
                # on GpSimd, whose queue exits the preamble ~0.8us before
                # Vector's, so warmup starts that much earlier.
                warm = const.tile([128, 512], BF16, name="warm")
                nc.gpsimd.memset(warm, 0.0)
                wps = psp.tile([128, 512], F32, tag="ps", name="wps")
                for i in range(5):
                    nc.tensor.matmul(wps, warm[:, 0:128], warm,
                                     start=True, stop=True)

                # ---- Phase A: Q projection for the OWN kv-half queries;
                # each chunk is bounced to DRAM for the pair AllGather.
                for c in range(XCH):
                    ps = [psp.tile([128, 512], F32, tag="ps",
                                   name=f"psq{c}_{e}") for e in range(ET)]
                    for dt in range(DT):
                        for e in range(ET):
                            nc.tensor.matmul(
                                ps[e], wqa[:, dt, e * 128:(e + 1) * 128],
                                xa[c][:, dt, :],
                                start=(dt == 0), stop=(dt == DT - 1))
                            # inline drain right after each e's stop-matmul
                            # so the banks free up before the next chunk
                            if dt == DT - 1:
                                nc.scalar.activation(
                                    qt_c[c][:, e, :], ps[e], Ident,
                                    bias=bqk_sb[:, e:e + 1], scale=1.0)
                    for g in range(ET // 2):
                        nc.sync.dma_start(
                            out=bass.AP(tensor=qown_c[c].tensor,
                                        offset=2 * g * 128 * 512,
                                        ap=[[512, 128], [128 * 512, 2],
                                            [1, 512]]),
                            in_=qt_c[c][:, 2 * g:2 * g + 2, :])
                    # per-chunk pairwise AllGather, launched as soon as
                    # this chunk's bounce is written; overlaps phase B
                    nc.gpsimd.collective_compute(
                        "AllGather", mybir.AluOpType.bypass,
                        replica_groups=[[0, 1], [2, 3], [4, 5], [6, 7]],
                        ins=[qown_c[c].opt()], outs=[qall_c[c].opt()])

                # wv loads follow the qown bounces; needed ~15us later
                for dt in range(DT):
                    dma_w(wva, wvt, dt)

                # peer half -> qt chunks 2,3.  The source half index is a
                # runtime register (1-parity): even cores read qall_c[1],
                # odd cores qall_c[0].  A single DMA per slice (vs a
                # cond-predicated pair) avoids WAW serialization.
                for c2 in range(2):
                    for e in range(ET):
                        src = qall_c[c2][bass.ts(not_parity, 1),
                                         e * 128:(e + 1) * 128, :]
                        nc.sync.dma_start(out=qt_c[2 + c2][:, e, :],
                                          in_=src)

                # ---- Phase B: K ----
                for cc in range(XCH):
                    ps = [psp.tile([128, 512], F32, tag="ps",
                                   name=f"psk{cc}_{e}") for e in range(ET)]
                    for dt in range(DT):
                        for e in range(ET):
                            nc.tensor.matmul(
                                ps[e], wka[:, dt, e * 128:(e + 1) * 128],
                                xa[cc][:, dt, :],
                                start=(dt == 0), stop=(dt == DT - 1))
                            if dt == DT - 1:
                                nc.scalar.activation(
                                    kt_c[cc][:, e, :], ps[e], Ident,
                                    bias=bqk_sb[:, ET + e:ET + e + 1],
                                    scale=1.0)

                # ---- Phase B: V (x-slices stationary, wv moving) ----
                for cc in range(XCH):
                    for j2 in range(4):
                        pv = [psp.tile([128, 512], F32, tag="ps",
                                       name=f"psv{cc}_{j2}_{ec}")
                              for ec in range(2)]
                        for dt in range(DT):
                            for ec in range(2):
                                nc.tensor.matmul(
                                    pv[ec],
                                    xa[cc][:, dt, j2 * 128:(j2 + 1) * 128],
                                    wva[:, dt, ec * 512:(ec + 1) * 512],
                                    start=(dt == 0), stop=(dt == DT - 1))
                        for ec in range(2):
                            nc.vector.tensor_copy(
                                v_c[cc][:, j2, ec * 512:(ec + 1) * 512],
                                pv[ec])

                # ================= Phase C: attention =================
                # l(blk): tree-sum the 8 PT tiles on DVE (f32 accumulate,
                # bf16 final), then a single ones^T matmul on the PE.
                # Emitted one block late so the exp+DVE dependencies are
                # long done (no PE stall); the lp tile shares the "ps" tag
                # so pool rotation keeps it inside the ST stream.
                # The DVE tree (7 serial adds, ~2.8us incl. the exp tail)
                # is emitted EARLY -- right after its ST block -- so it
                # runs while the PE is still on ST/accT matmuls; the lp
                # matmul is emitted several chains later and never waits.
                lsb_t = {}

                def emit_l_tree(blk):
                    lts = ltp.tile([128, 512], F32, tag="lt",
                                   name=f"lt{blk}")
                    lsb = ltp.tile([128, 512], BF16, tag="lsb",
                                   name=f"lsb{blk}")
                    nc.vector.tensor_add(lts, pt_b[blk][:, 0, :],
                                         pt_b[blk][:, 1, :])
                    for j in range(2, 7):
                        nc.vector.tensor_add(lts, lts, pt_b[blk][:, j, :])
                    nc.vector.tensor_add(lsb, lts, pt_b[blk][:, 7, :])
                    lsb_t[blk] = lsb

                def emit_l_mat(blk):
                    lp = psp.tile([128, 512], F32, tag="ps",
                                  name=f"lp{blk}")
                    nc.tensor.matmul(lp[0:1, :], ones_sb, lsb_t[blk],
                                     start=True, stop=True)
                    l_st = lst.tile([1, 512], F32, tag="l",
                                    name=f"lst{blk}")
                    nc.vector.tensor_copy(l_st, lp[0:1, :])
                    nc.sync.dma_start(
                        out=l_d[blk * 512:(blk + 1) * 512], in_=l_st)

                # ---- ST = K @ Q^T, PT = exp(ST/32); queries in LOCAL
                # order: blocks 0,1 = own half (no collective dep),
                # blocks 2,3 = peer half.
                def st_block(blk):
                    for j in range(8):
                        cc, jj = divmod(j, 4)
                        sp = psp.tile([128, 512], F32, tag="ps",
                                      name=f"sp{blk}_{j}")
                        for e in range(ET):
                            nc.tensor.matmul(
                                sp, kt_c[cc][:, e, jj * 128:(jj + 1) * 128],
                                qt_c[blk][:, e, :],
                                start=(e == 0), stop=(e == ET - 1))
                        nc.scalar.activation(
                            pt_b[blk][:, j, :], sp, Exp,
                            bias=0.0, scale=float(1.0 / 32.0))

                # ---- accT = V^T-slices @ PT (one query block) ----
                def acc_chain(e, blk):
                    if e == ET - 1 and blk == NCH - 1:
                        # final tile: two uneven half-free chains so the
                        # tail pipeline only carries a [128,128] piece
                        # after the very last PE op
                        for (q0, q1) in ((0, 384), (384, 512)):
                            avh = psp.tile([128, q1 - q0], F32,
                                           tag="ps", name=f"avf{q0}")
                            for j in range(8):
                                cc, jj = divmod(j, 4)
                                nc.tensor.matmul(
                                    avh,
                                    v_c[cc][:, jj, e * 128:(e + 1) * 128],
                                    pt_b[blk][:, j, q0:q1],
                                    start=(j == 0), stop=(j == 7))
                            sth = stg.tile([128, q1 - q0], BF16,
                                           tag="stgh", name=f"stf{q0}")
                            nc.vector.tensor_copy(sth, avh)
                            nc.sync.dma_start(
                                out=accT_d[
                                    e * 128:(e + 1) * 128,
                                    blk * 512 + q0:blk * 512 + q1],
                                in_=sth)
                        return
                    av = psp.tile([128, 512], F32, tag="ps",
                                  name=f"av{e}_{blk}")
                    for j in range(8):
                        cc, jj = divmod(j, 4)
                        nc.tensor.matmul(
                            av,
                            v_c[cc][:, jj, e * 128:(e + 1) * 128],
                            pt_b[blk][:, j, :],
                            start=(j == 0), stop=(j == 7))
                    st_t = stg.tile([128, 512], BF16, tag="stg",
                                    name=f"acc{e}_{blk}")
                    nc.vector.tensor_copy(st_t, av)
                    nc.sync.dma_start(
                        out=accT_d[e * 128:(e + 1) * 128,
                                   blk * 512:(blk + 1) * 512],
                        in_=st_t)

                # ST blocks 2,3 (peer queries) are deferred until after
                # the accT work for blocks 0,1: the collective+gather-in
                # path gets ~57us/~84us of slack instead of ~28us, which
                # rides out cross-core skew on the AllGather rendezvous.
                # Each lp(b) is emitted a few chains after its exps so the
                # serial DVE tree (~2.8us) never stalls the PE.
                st_block(0)
                st_block(1)
                emit_l(0)
                for e in range(ET):
                    acc_chain(e, 0)
                emit_l(1)
                for e in range(ET):
                    acc_chain(e, 1)
                st_block(2)
                acc_chain(0, 2)
                acc_chain(1, 2)
                emit_l(2)
                for e in range(2, ET):
                    acc_chain(e, 2)
                st_block(3)
                acc_chain(0, 3)
                acc_chain(1, 3)
                emit_l(3)
                for e in range(2, ET):
                    acc_chain(e, 3)

    nc.compile()
    return nc


def _get_compiled():
    global _compiled
    if _compiled is None:
        _compiled = _build()
    return _compiled


def run_sharded(inputs, **run_kwargs):
    """Build per-core in_maps, run SPMD, return BassKernelResults."""
    x = np.ascontiguousarray(inputs["x"], dtype=np.float32)
    Wq = np.asarray(inputs["Wq"], dtype=np.float32)
    Wk = np.asarray(inputs["Wk"], dtype=np.float32)
    Wv = np.asarray(inputs["Wv"], dtype=np.float32)
    bq = np.asarray(inputs["bq"], dtype=np.float32)
    bk = np.asarray(inputs["bk"], dtype=np.float32)

    nc = _get_compiled()

    wqt = np.ascontiguousarray(Wq.T).astype(BF)
    wkt = np.ascontiguousarray(Wk.T).astype(BF)
    wvt = np.ascontiguousarray(Wv.T).astype(BF)
    ones = np.ones((128, 1), dtype=np.float32).astype(BF)
    # packed per-partition biases: bqk[p, e] = bq[e*128+p],
    # bqk[p, 8+e] = bk[e*128+p]
    bqkm = np.concatenate([bq.reshape(ET, 128).T, bk.reshape(ET, 128).T],
                          axis=1).astype(np.float32)
    bqkm = np.ascontiguousarray(bqkm)

    in_maps = []
    for core in range(N_CORES):
        b, h = divmod(core, 2)
        xhb = x[b].T[:, h * H:(h + 1) * H]            # [D, H] own kv-half
        in_maps.append(dict(xh=np.ascontiguousarray(xhb).astype(BF),
                            wqt=wqt, wkt=wkt, wvt=wvt,
                            bqk=bqkm, ones=ones))

    return run_bass_kernel_spmd(nc, in_maps, core_ids=list(range(N_CORES)),
                                **run_kwargs)


def kernel(**inputs):
    bv = np.asarray(inputs["bv"], dtype=np.float64)
    res = run_sharded(inputs)

    out = np.empty((B, S, D), dtype=np.float32)
    for b in range(B):
        r0 = res.results[b * 2]
        r1 = res.results[b * 2 + 1]
        a0 = np.asarray(r0["accT_d"], dtype=np.float64)       # [D, S]
        a1 = np.asarray(r1["accT_d"], dtype=np.float64)
        # each core's output query order is [own half | peer half];
        # odd cores' halves are swapped relative to natural order
        a1 = np.concatenate([a1[:, H:], a1[:, :H]], axis=1)
        l0 = np.asarray(r0["l_d"], dtype=np.float64)
        l1 = np.asarray(r1["l_d"], dtype=np.float64)
        l1 = np.concatenate([l1[H:], l1[:H]])
        num = a0.T + a1.T
        den = (l0 + l1)[:, None]
        out[b] = (num / den + bv[None, :]).astype(np.float32)
    return out


# revision 29
# speedup vs baseline: 1.0072x; 1.0072x over previous
"""Single-head attention (B=4, S=2048, D=1024) on 8 trn2 NeuronCores.

Sharding: core = batch*2 + kv_half.  Each core receives ONLY its kv-half
of x^T (xh = x[b].T[:, half]) and computes
  Qown = xh @ Wq^T + bq        (its 1024 own-half queries)
  K    = xh @ Wk^T + bk        (its 1024 keys)
  V    = xh @ Wv^T             (bv folded in on host)
The pair cores exchange Q^T halves with a pairwise HBM AllGather (2MB)
that overlaps the K/V projections; each core then attends all 2048
queries against its kv-half:
  ST = K @ Q^T, PT = exp(ST/32), l = ones^T @ sum_j PT_j, accT = V' @ PT
Queries are processed in LOCAL order ([own half | peer half]) so the ST
pass starts without waiting on the collective; the host swaps the odd
cores' output halves back and merges:
  out[b] = (acc0 + acc1) / (l0 + l1) + bv.

All tensors are bf16 on device (fp32 PSUM); bf16 matmuls run at full PE
rate and everything stays SBUF-resident.  DMAs are issued on the sync
ring only (ACT-ring DMAs wedge the device) and are emitted in deadline
order — the SP engine processes them serially at ~185GB/s, so program
order is the schedule.  The peer-half gather-in DMAs read
qall_d[1 - parity] through a runtime register index (bass.ts) so one
NEFF serves both pair members.

Perf notes vs the first working version (224.2us):
 - weight DMAs are one [128,1024] burst per row-block (2KB/partition);
 - bq/bk ride in a single packed [128,16] f32 DMA placed late enough
   not to delay phase-A x/w tiles but before the first PSUM drain;
 - only 5 PE warmup matmuls: phase A itself ramps the clock inside the
   DMA-starved window instead of junk work delaying it;
 - l(blk) = ones^T @ (tree-sum of the 8 PT tiles on DVE): 1 PE matmul
   per block instead of 8 (-14K PE cycles);
 - one PSUM pool for all phases (no pool-transition barrier);
 - the last accT drain is split in half so the tail cast+DMA pipeline
   drains in smaller quanta.
"""

import sys
import numpy as np

for _p in ("/root/.axon_site/_ro/trn_rl_repo", "/opt/trn_rl_repo"):
    if _p not in sys.path:
        sys.path.append(_p)

import ml_dtypes
import concourse.bass as bass
import concourse.tile as tile
from concourse import bacc, mybir
from concourse.bass_utils import run_bass_kernel_spmd

F32 = mybir.dt.float32
BF16 = mybir.dt.bfloat16
BF = ml_dtypes.bfloat16

B, S, D = 4, 2048, 1024
H = S // 2          # kv-half size (1024)
DT = D // 128       # 8 contraction tiles
ET = D // 128       # 8 output-dim tiles
XCH = H // 512      # 2 column chunks of xh
NCH = S // 512      # 4 query chunks
N_CORES = 8

_compiled = None


def _build():
    nc = bacc.Bacc("TRN2", target_bir_lowering=False, debug=False,
                   num_devices=N_CORES)

    xh = nc.dram_tensor("xh", [D, H], BF16, kind="ExternalInput").ap()
    wqt = nc.dram_tensor("wqt", [D, D], BF16, kind="ExternalInput").ap()
    wkt = nc.dram_tensor("wkt", [D, D], BF16, kind="ExternalInput").ap()
    wvt = nc.dram_tensor("wvt", [D, D], BF16, kind="ExternalInput").ap()
    bqk = nc.dram_tensor("bqk", [128, 2 * ET], F32, kind="ExternalInput").ap()
    ones = nc.dram_tensor("ones", [128, 1], BF16, kind="ExternalInput").ap()

    accT_d = nc.dram_tensor("accT_d", [D, S], BF16, kind="ExternalOutput").ap()
    l_d = nc.dram_tensor("l_d", [S], F32, kind="ExternalOutput").ap()

    Ident = mybir.ActivationFunctionType.Identity
    Exp = mybir.ActivationFunctionType.Exp

    with tile.TileContext(nc) as tc:
        with (
            tc.tile_pool(name="const", bufs=1) as const,
            tc.tile_pool(name="wpool", bufs=1) as wpool,
            tc.tile_pool(name="xpool", bufs=1) as xpool,
            tc.tile_pool(name="qtp", bufs=1) as qtp,
            tc.tile_pool(name="ktp", bufs=1) as ktp,
            tc.tile_pool(name="vvp", bufs=1) as vvp,
            tc.tile_pool(name="ptp", bufs=1) as ptp,
            tc.tile_pool(name="ltp", bufs=2) as ltp,
            tc.tile_pool(name="stg", bufs=4) as stg,
            tc.tile_pool(name="lst", bufs=2) as lst,
        ):
            ones_sb = const.tile([128, 1], BF16, name="ones_sb")
            bqk_sb = const.tile([128, 2 * ET], F32, name="bqk_sb")

            wqa = wpool.tile([128, DT, D], BF16, name="wqa")
            wka = wpool.tile([128, DT, D], BF16, name="wka")
            wva = wpool.tile([128, DT, D], BF16, name="wva")
            xa = [xpool.tile([128, DT, 512], BF16, name=f"xa{c}")
                  for c in range(XCH)]
            qt_c = [qtp.tile([128, ET, 512], BF16, name=f"qt{c}")
                    for c in range(NCH)]
            kt_c = [ktp.tile([128, ET, 512], BF16, name=f"kt{cc}")
                    for cc in range(XCH)]
            v_c = [vvp.tile([128, 4, D], BF16, name=f"v{cc}")
                   for cc in range(XCH)]
            pt_b = [ptp.tile([128, 8, 512], BF16, name=f"pt{blk}")
                    for blk in range(NCH)]

            # DRAM bounce buffers for the pairwise Q AllGather, split per
            # 512-query chunk so the first exchange starts ~15us earlier
            with tc.tile_pool(name="dram", bufs=1, space="DRAM") as dram:
                qown_c = [dram.tile([D, 512], BF16, name=f"qown{c}")
                          for c in range(XCH)]
                qall_c = [dram.tile([2, D, 512], BF16, name=f"qall{c}")
                          for c in range(XCH)]

            # core parity (runtime register) for the predicated gather-in
            parity = nc.sync.partition_id() & 1
            not_parity = 1 - parity

            # ---- input DMAs in deadline order (transfers run at
            # ~180GB/s effective aggregate regardless of burst size, so
            # per-row-block granularity minimizes first-use latency;
            # program order IS the schedule) ----
            def dma_w(dst, src, dt):
                nc.sync.dma_start(
                    out=dst[:, dt, :],
                    in_=src[dt * 128:(dt + 1) * 128, :])

            def dma_x(c, dt):
                nc.sync.dma_start(
                    out=xa[c][:, dt, :],
                    in_=xh[dt * 128:(dt + 1) * 128, c * 512:(c + 1) * 512])

            # first-needed pieces lead the stream in consumption order;
            # the packed const DMA slots in after dt5 (needed at the first
            # PSUM drain, which happens at dt7 of chunk 0).  wq's dt0 rows
            # go in two column-halves so the very first matmul chains
            # (e0-3 of dt0) only wait on a 128KB piece, not 256KB.
            nc.sync.dma_start(out=wqa[:, 0, 0:512],
                              in_=wqt[0:128, 0:512])
            dma_x(0, 0)
            nc.sync.dma_start(out=wqa[:, 0, 512:1024],
                              in_=wqt[0:128, 512:1024])
            for dt in range(1, 6):
                dma_w(wqa, wqt, dt)
                dma_x(0, dt)
            nc.sync.dma_start(out=ones_sb, in_=ones)
            nc.sync.dma_start(out=bqk_sb, in_=bqk)
            for dt in range(6, DT):
                dma_w(wqa, wqt, dt)
                dma_x(0, dt)
            for dt in range(DT):
                dma_x(1, dt)
            for dt in range(DT):
                dma_w(wka, wkt, dt)

            with tc.tile_pool(name="ps", bufs=8, space="PSUM") as psp:
                # ---- PE warmup: junk matmuls kick the PE clock ramp
                # while the first input DMAs land; phase A then continues
                # the ramp inside its DMA-paced window.  The memset runs
                # on GpSimd, whose queue exits the preamble ~0.8us before
                # Vector's, so warmup starts that much earlier.
                warm = const.tile([128, 512], BF16, name="warm")
                nc.gpsimd.memset(warm, 0.0)
                wps = psp.tile([128, 512], F32, tag="ps", name="wps")
                for i in range(5):
                    nc.tensor.matmul(wps, warm[:, 0:128], warm,
                                     start=True, stop=True)

                # ---- Phase A: Q projection for the OWN kv-half queries;
                # each chunk is bounced to DRAM for the pair AllGather.
                for c in range(XCH):
                    ps = [psp.tile([128, 512], F32, tag="ps",
                                   name=f"psq{c}_{e}") for e in range(ET)]
                    for dt in range(DT):
                        for e in range(ET):
                            nc.tensor.matmul(
                                ps[e], wqa[:, dt, e * 128:(e + 1) * 128],
                                xa[c][:, dt, :],
                                start=(dt == 0), stop=(dt == DT - 1))
                            # inline drain right after each e's stop-matmul
                            # so the banks free up before the next chunk
                            if dt == DT - 1:
                                nc.scalar.activation(
                                    qt_c[c][:, e, :], ps[e], Ident,
                                    bias=bqk_sb[:, e:e + 1], scale=1.0)
                    for g in range(ET // 2):
                        nc.sync.dma_start(
                            out=bass.AP(tensor=qown_c[c].tensor,
                                        offset=2 * g * 128 * 512,
                                        ap=[[512, 128], [128 * 512, 2],
                                            [1, 512]]),
                            in_=qt_c[c][:, 2 * g:2 * g + 2, :])
                    # per-chunk pairwise AllGather, launched as soon as
                    # this chunk's bounce is written; overlaps phase B
                    nc.gpsimd.collective_compute(
                        "AllGather", mybir.AluOpType.bypass,
                        replica_groups=[[0, 1], [2, 3], [4, 5], [6, 7]],
                        ins=[qown_c[c].opt()], outs=[qall_c[c].opt()])

                # wv loads follow the qown bounces; needed ~15us later
                for dt in range(DT):
                    dma_w(wva, wvt, dt)

                # peer half -> qt chunks 2,3.  The source half index is a
                # runtime register (1-parity): even cores read qall_c[1],
                # odd cores qall_c[0].  A single DMA per slice (vs a
                # cond-predicated pair) avoids WAW serialization.
                for c2 in range(2):
                    for e in range(ET):
                        src = qall_c[c2][bass.ts(not_parity, 1),
                                         e * 128:(e + 1) * 128, :]
                        nc.sync.dma_start(out=qt_c[2 + c2][:, e, :],
                                          in_=src)

                # ---- Phase B: K ----
                for cc in range(XCH):
                    ps = [psp.tile([128, 512], F32, tag="ps",
                                   name=f"psk{cc}_{e}") for e in range(ET)]
                    for dt in range(DT):
                        for e in range(ET):
                            nc.tensor.matmul(
                                ps[e], wka[:, dt, e * 128:(e + 1) * 128],
                                xa[cc][:, dt, :],
                                start=(dt == 0), stop=(dt == DT - 1))
                            if dt == DT - 1:
                                nc.scalar.activation(
                                    kt_c[cc][:, e, :], ps[e], Ident,
                                    bias=bqk_sb[:, ET + e:ET + e + 1],
                                    scale=1.0)

                # ---- Phase B: V (x-slices stationary, wv moving) ----
                for cc in range(XCH):
                    for j2 in range(4):
                        pv = [psp.tile([128, 512], F32, tag="ps",
                                       name=f"psv{cc}_{j2}_{ec}")
                              for ec in range(2)]
                        for dt in range(DT):
                            for ec in range(2):
                                nc.tensor.matmul(
                                    pv[ec],
                                    xa[cc][:, dt, j2 * 128:(j2 + 1) * 128],
                                    wva[:, dt, ec * 512:(ec + 1) * 512],
                                    start=(dt == 0), stop=(dt == DT - 1))
                        for ec in range(2):
                            nc.vector.tensor_copy(
                                v_c[cc][:, j2, ec * 512:(ec + 1) * 512],
                                pv[ec])

                # ================= Phase C: attention =================
                # l(blk): tree-sum the 8 PT tiles on DVE (f32 accumulate,
                # bf16 final), then a single ones^T matmul on the PE.
                # Emitted one block late so the exp+DVE dependencies are
                # long done (no PE stall); the lp tile shares the "ps" tag
                # so pool rotation keeps it inside the ST stream.
                # The DVE tree (7 serial adds, ~2.8us incl. the exp tail)
                # is emitted EARLY -- right after its ST block -- so it
                # runs while the PE is still on ST/accT matmuls; the lp
                # matmul is emitted several chains later and never waits.
                lsb_t = {}

                def emit_l_tree(blk):
                    lts = ltp.tile([128, 512], F32, tag="lt",
                                   name=f"lt{blk}")
                    lsb = ltp.tile([128, 512], BF16, tag="lsb",
                                   name=f"lsb{blk}")
                    nc.vector.tensor_add(lts, pt_b[blk][:, 0, :],
                                         pt_b[blk][:, 1, :])
                    for j in range(2, 7):
                        nc.vector.tensor_add(lts, lts, pt_b[blk][:, j, :])
                    nc.vector.tensor_add(lsb, lts, pt_b[blk][:, 7, :])
                    lsb_t[blk] = lsb

                def emit_l_mat(blk):
                    lp = psp.tile([128, 512], F32, tag="ps",
                                  name=f"lp{blk}")
                    nc.tensor.matmul(lp[0:1, :], ones_sb, lsb_t[blk],
                                     start=True, stop=True)
                    l_st = lst.tile([1, 512], F32, tag="l",
                                    name=f"lst{blk}")
                    nc.vector.tensor_copy(l_st, lp[0:1, :])
                    nc.sync.dma_start(
                        out=l_d[blk * 512:(blk + 1) * 512], in_=l_st)

                # ---- ST = K @ Q^T, PT = exp(ST/32); queries in LOCAL
                # order: blocks 0,1 = own half (no collective dep),
                # blocks 2,3 = peer half.
                def st_block(blk):
                    for j in range(8):
                        cc, jj = divmod(j, 4)
                        sp = psp.tile([128, 512], F32, tag="ps",
                                      name=f"sp{blk}_{j}")
                        for e in range(ET):
                            nc.tensor.matmul(
                                sp, kt_c[cc][:, e, jj * 128:(jj + 1) * 128],
                                qt_c[blk][:, e, :],
                                start=(e == 0), stop=(e == ET - 1))
                        nc.scalar.activation(
                            pt_b[blk][:, j, :], sp, Exp,
                            bias=0.0, scale=float(1.0 / 32.0))

                # ---- accT = V^T-slices @ PT (one query block) ----
                def acc_chain(e, blk):
                    if e == ET - 1 and blk == NCH - 1:
                        # final tile: two uneven half-free chains so the
                        # tail pipeline only carries a [128,128] piece
                        # after the very last PE op
                        for (q0, q1) in ((0, 384), (384, 512)):
                            avh = psp.tile([128, q1 - q0], F32,
                                           tag="ps", name=f"avf{q0}")
                            for j in range(8):
                                cc, jj = divmod(j, 4)
                                nc.tensor.matmul(
                                    avh,
                                    v_c[cc][:, jj, e * 128:(e + 1) * 128],
                                    pt_b[blk][:, j, q0:q1],
                                    start=(j == 0), stop=(j == 7))
                            sth = stg.tile([128, q1 - q0], BF16,
                                           tag="stgh", name=f"stf{q0}")
                            nc.vector.tensor_copy(sth, avh)
                            nc.sync.dma_start(
                                out=accT_d[
                                    e * 128:(e + 1) * 128,
                                    blk * 512 + q0:blk * 512 + q1],
                                in_=sth)
                        return
                    av = psp.tile([128, 512], F32, tag="ps",
                                  name=f"av{e}_{blk}")
                    for j in range(8):
                        cc, jj = divmod(j, 4)
                        nc.tensor.matmul(
                            av,
                            v_c[cc][:, jj, e * 128:(e + 1) * 128],
                            pt_b[blk][:, j, :],
                            start=(j == 0), stop=(j == 7))
                    st_t = stg.tile([128, 512], BF16, tag="stg",
                                    name=f"acc{e}_{blk}")
                    nc.vector.tensor_copy(st_t, av)
                    nc.sync.dma_start(
                        out=accT_d[e * 128:(e + 1) * 128,
                                   blk * 512:(blk + 1) * 512],
                        in_=st_t)

                # ST blocks 2,3 (peer queries) are deferred until after
                # the accT work for blocks 0,1: the collective+gather-in
                # path gets ~57us/~84us of slack instead of ~28us, which
                # rides out cross-core skew on the AllGather rendezvous.
                # Each lp(b) is emitted a few chains after its exps so the
                # serial DVE tree (~2.8us) never stalls the PE.
                st_block(0)
                emit_l_tree(0)
                st_block(1)
                emit_l_tree(1)
                emit_l_mat(0)
                for e in range(ET):
                    acc_chain(e, 0)
                emit_l_mat(1)
                for e in range(ET):
                    acc_chain(e, 1)
                st_block(2)
                emit_l_tree(2)
                for e in range(3):
                    acc_chain(e, 2)
                emit_l_mat(2)
                for e in range(3, ET):
                    acc_chain(e, 2)
                st_block(3)
                emit_l_tree(3)
                for e in range(3):
                    acc_chain(e, 3)
                emit_l_mat(3)
                for e in range(3, ET):
                    acc_chain(e, 3)

    nc.compile()
    return nc


def _get_compiled():
    global _compiled
    if _compiled is None:
        _compiled = _build()
    return _compiled


def run_sharded(inputs, **run_kwargs):
    """Build per-core in_maps, run SPMD, return BassKernelResults."""
    x = np.ascontiguousarray(inputs["x"], dtype=np.float32)
    Wq = np.asarray(inputs["Wq"], dtype=np.float32)
    Wk = np.asarray(inputs["Wk"], dtype=np.float32)
    Wv = np.asarray(inputs["Wv"], dtype=np.float32)
    bq = np.asarray(inputs["bq"], dtype=np.float32)
    bk = np.asarray(inputs["bk"], dtype=np.float32)

    nc = _get_compiled()

    wqt = np.ascontiguousarray(Wq.T).astype(BF)
    wkt = np.ascontiguousarray(Wk.T).astype(BF)
    wvt = np.ascontiguousarray(Wv.T).astype(BF)
    ones = np.ones((128, 1), dtype=np.float32).astype(BF)
    # packed per-partition biases: bqk[p, e] = bq[e*128+p],
    # bqk[p, 8+e] = bk[e*128+p]
    bqkm = np.concatenate([bq.reshape(ET, 128).T, bk.reshape(ET, 128).T],
                          axis=1).astype(np.float32)
    bqkm = np.ascontiguousarray(bqkm)

    in_maps = []
    for core in range(N_CORES):
        b, h = divmod(core, 2)
        xhb = x[b].T[:, h * H:(h + 1) * H]            # [D, H] own kv-half
        in_maps.append(dict(xh=np.ascontiguousarray(xhb).astype(BF),
                            wqt=wqt, wkt=wkt, wvt=wvt,
                            bqk=bqkm, ones=ones))

    return run_bass_kernel_spmd(nc, in_maps, core_ids=list(range(N_CORES)),
                                **run_kwargs)


def kernel(**inputs):
    bv = np.asarray(inputs["bv"], dtype=np.float64)
    res = run_sharded(inputs)

    out = np.empty((B, S, D), dtype=np.float32)
    for b in range(B):
        r0 = res.results[b * 2]
        r1 = res.results[b * 2 + 1]
        a0 = np.asarray(r0["accT_d"], dtype=np.float64)       # [D, S]
        a1 = np.asarray(r1["accT_d"], dtype=np.float64)
        # each core's output query order is [own half | peer half];
        # odd cores' halves are swapped relative to natural order
        a1 = np.concatenate([a1[:, H:], a1[:, :H]], axis=1)
        l0 = np.asarray(r0["l_d"], dtype=np.float64)
        l1 = np.asarray(r1["l_d"], dtype=np.float64)
        l1 = np.concatenate([l1[H:], l1[:H]])
        num = a0.T + a1.T
        den = (l0 + l1)[:, None]
        out[b] = (num / den + bv[None, :]).astype(np.float32)
    return out
